# revision 14
# baseline (speedup 1.0000x reference)
"""BitLinear-STE forward on 8 Trainium2 NeuronCores.

Reference computes y = x @ sign(W).T with x:(4,2048,4096) f32, W:(4096,4096) f32.
Forward-only, so the STE proxy reduces to a plain matmul against sign(W).

Strategy (data parallel over rows, per the sharding hint):
  - host: q = sign(W) cast to fp16 (exact, values are +-1) and transposed to
    [in, out]; x cast to fp16 (rounding error ~2e-4 on the output) and
    transposed per-core to [in, rows/8].  Transposes happen on host because
    the TensorE contracts over the partition dim, which must be in_features
    for both operands, while in_features is the contiguous axis of both
    DRAM tensors.
  - each core computes its 1024-row slice of y = xT.T @ wqT with fp32
    accumulation in PSUM, streaming W (32 MiB fp16) once from HBM while the
    x shard (8 MiB fp16) stays SBUF-resident.  Loads are 256 KiB pieces
    chained into serial lanes in first-use order so the PE starts ~10us in
    and then streams 2048 N=512 matmuls back-to-back at ~217 ns each
    (hardware peak is ~216 ns: 512 cols / 2.4 GHz + NX issue overhead).
  - host concatenates the 8 row-slices.

Measured on trn2: ~462 us per core (roofline: 437 us of fp16 matmul),
2-norm relative error ~2.1e-4 vs the fp32 reference.
"""

import numpy as np

import concourse.mybir as mybir
import concourse.tile as tile
from concourse import bacc
from concourse.bass_utils import run_bass_kernel_spmd
from concourse.tile import add_dep_helper

N_CORES = 8
P = 128
IN_F = 4096
OUT_F = 4096
ROWS = 4 * 2048
ROWS_PER_CORE = ROWS // N_CORES      # 1024
I_TILES = IN_F // P                  # 32
O_BLK = 512
O_BLKS = OUT_F // O_BLK              # 8
S_TILES = ROWS_PER_CORE // P         # 8

F16 = mybir.dt.float16
F32 = mybir.dt.float32

_NC_CACHE = {}


def _build_nc(in_f=IN_F, out_f=OUT_F, rows_per_core=ROWS_PER_CORE):
    i_tiles = in_f // P
    o_blks = out_f // O_BLK
    s_tiles = rows_per_core // P
    xg = 8 if i_tiles % 8 == 0 else 1       # DMA split granularity
    wg = 4 if i_tiles % 4 == 0 else 1

    nc = bacc.Bacc(None, target_bir_lowering=False)
    xt = nc.dram_tensor("xt", (in_f, rows_per_core), F16, kind="ExternalInput")
    wt = nc.dram_tensor("wt", (in_f, out_f), F16, kind="ExternalInput")
    y = nc.dram_tensor("y", (rows_per_core, out_f), F32, kind="ExternalOutput")

    xt_v = xt.rearrange("(ih p) s -> p ih s", p=P)   # [128, i_tiles, rows]
    wt_v = wt.rearrange("(ih p) o -> p ih o", p=P)   # [128, i_tiles, out_f]
    y_v = y.rearrange("(st p) o -> st p o", p=P)     # [s_tiles, 128, out_f]

    wq = 2                                  # i-tiles per w quarter-DMA (256 KiB)
    w_quarters = i_tiles // wq
    LANES = 8

    with tile.TileContext(nc) as tc:
        with (
            tc.tile_pool(name="xp", bufs=1) as xp,
            tc.tile_pool(name="wp", bufs=2) as wp,
            tc.tile_pool(name="op", bufs=4) as op,
            tc.tile_pool(name="pp", bufs=1, space="PSUM") as pp,
        ):
            # --- startup pipelining -------------------------------------
            # DMAs issued together fair-share HBM bandwidth, so an unordered
            # prefetch makes the first matmul wait for everything (~35us).
            # Instead every load is a 256 KiB piece, chained into LANES
            # serial chains in exact first-use order; o-block 0 runs
            # i-outer across the 8 PSUM banks so the PE starts as soon as
            # the first pieces land and streams behind the DMA wavefront.
            lane_tails = [None] * LANES
            n_item = 0
            head_dma = None  # first critical piece; lane heads chain off it

            def chained_dma(dst, src):
                nonlocal n_item
                lane = n_item % LANES
                d = nc.scalar.dma_start(dst, src)
                dep = lane_tails[lane] if lane_tails[lane] is not None else head_dma
                if dep is not None:
                    add_dep_helper(d.ins, dep.ins, reason="load lane")
                lane_tails[lane] = d
                n_item += 1
                return d

            # per-i-tile x tiles; allocated up front, loaded in need order
            x_tiles = [
                xp.tile([P, rows_per_core], F16, tag=f"x{i}", name=f"x{i}")
                for i in range(i_tiles)
            ]

            # PE warm-up: ~8 dummy matmuls while the first loads are in
            # flight flip the HAM clock gate (1.2 -> 2.4 GHz takes ~3.4us
            # of sustained PE activity) so the real stream starts warm.
            dm = op.tile([P, O_BLK], F16, tag="warm", name="warm")
            nc.any.memset(dm, 0.0)
            dps = pp.tile([P, O_BLK], F32, tag="ps0", name="warmps")
            for _ in range(8):
                nc.tensor.matmul(dps, dm[:, :P], dm, start=True, stop=True)

            def load_w_quarter(w_tiles, q, osl, chained):
                wtile = wp.tile([P, wq, O_BLK], F16, tag=f"w{q}", name=f"w{q}")
                src = wt_v[:, q * wq : (q + 1) * wq, osl]
                if chained:
                    chained_dma(wtile, src)
                else:
                    nc.scalar.dma_start(wtile, src)
                w_tiles.append(wtile)

            for ob in range(o_blks):
                osl = slice(ob * O_BLK, (ob + 1) * O_BLK)
                w_tiles = []
                if ob == 0:
                    # Critical head: the first matmuls need only w[i=0] and
                    # the first half of x[i=0] — ship those two 128 KiB
                    # pieces alone at full bandwidth on nc.sync; everything
                    # else chains behind the w head in LANES serial lanes.
                    half = rows_per_core // 2
                    wtile = wp.tile([P, wq, O_BLK], F16, tag="w0", name="w0")
                    head_dma = nc.sync.dma_start(wtile[:, 0:1, :], wt_v[:, 0:1, osl])
                    nc.sync.dma_start(x_tiles[0][:, :half], xt_v[:, 0, :half])
                    w_tiles.append(wtile)
                    # rest of the head tiles, then pieces in first-use order
                    chained_dma(x_tiles[0][:, half:], xt_v[:, 0, half:])
                    chained_dma(wtile[:, 1:2, :], wt_v[:, 1:2, osl])
                    chained_dma(x_tiles[1], xt_v[:, 1, :])
                    for q in range(1, w_quarters):
                        load_w_quarter(w_tiles, q, osl, chained=True)
                        for i in (wq * q, wq * q + 1):
                            chained_dma(x_tiles[i], xt_v[:, i, :])
                elif ob == 1:
                    # keep feeding the lanes; arrives during ob0 compute
                    for q in range(w_quarters):
                        load_w_quarter(w_tiles, q, osl, chained=True)
                else:
                    # paced naturally by slot reuse (bufs=2 per tag)
                    for q in range(w_quarters):
                        load_w_quarter(w_tiles, q, osl, chained=False)

                if ob == 0:
                    # i-outer: all 8 s-tiles accumulate in parallel banks,
                    # consuming input pieces in arrival order
                    pss = [
                        pp.tile([P, O_BLK], F32, tag=f"ps{st}", name=f"ps0_{st}")
                        for st in range(s_tiles)
                    ]
                    for i in range(i_tiles):
                        for st in range(s_tiles):
                            nc.tensor.matmul(
                                pss[st],
                                x_tiles[i][:, st * P : (st + 1) * P],
                                w_tiles[i // wq][:, i % wq, :],
                                start=(i == 0),
                                stop=(i == i_tiles - 1),
                            )
                    for st in range(s_tiles):
                        o_sb = op.tile([P, O_BLK], F32)
                        nc.vector.tensor_copy(o_sb, pss[st])
                        nc.sync.dma_start(y_v[st, :, osl], o_sb)
                else:
                    for st in range(s_tiles):
                        ps = pp.tile([P, O_BLK], F32, tag=f"ps{st}")
                        for i in range(i_tiles):
                            nc.tensor.matmul(
                                ps,
                                x_tiles[i][:, st * P : (st + 1) * P],
                                w_tiles[i // wq][:, i % wq, :],
                                start=(i == 0),
                                stop=(i == i_tiles - 1),
                            )
                        o_sb = op.tile([P, O_BLK], F32)
                        nc.vector.tensor_copy(o_sb, ps)
                        nc.sync.dma_start(y_v[st, :, osl], o_sb)
    nc.finalize()
    return nc


def _get_nc():
    if "nc" not in _NC_CACHE:
        _NC_CACHE["nc"] = _build_nc()
    return _NC_CACHE["nc"]


def _prep_inputs(x, weight):
    x2 = np.ascontiguousarray(x, dtype=np.float32).reshape(ROWS, IN_F).astype(np.float16)
    wq = np.sign(weight.astype(np.float32)).astype(np.float16)
    wt = np.ascontiguousarray(wq.T)  # [in, out]
    in_maps = []
    for c in range(N_CORES):
        xs = np.ascontiguousarray(x2[c * ROWS_PER_CORE : (c + 1) * ROWS_PER_CORE].T)
        in_maps.append({"xt": xs, "wt": wt})
    return in_maps


def _run(x, weight, trace=False, trace_cores=None):
    in_maps = _prep_inputs(x, weight)
    res = run_bass_kernel_spmd(
        _get_nc(),
        in_maps,
        core_ids=list(range(N_CORES)),
        trace=trace,
        trace_cores=trace_cores,
    )
    out = np.concatenate([res.results[c]["y"] for c in range(N_CORES)], axis=0)
    return out.reshape(4, 2048, OUT_F), res


def kernel(x, weight):
    out, _ = _run(x, weight, trace=False)
    return out


# revision 16
# speedup vs baseline: 1.0020x; 1.0020x over previous
"""BitLinear-STE forward on 8 Trainium2 NeuronCores.

Reference computes y = x @ sign(W).T with x:(4,2048,4096) f32, W:(4096,4096) f32.
Forward-only, so the STE proxy reduces to a plain matmul against sign(W).

Strategy (data parallel over rows, per the sharding hint):
  - host: q = sign(W) cast to fp16 (exact, values are +-1) and transposed to
    [in, out]; x cast to fp16 (rounding error ~2e-4 on the output) and
    transposed per-core to [in, rows/8].  Transposes happen on host because
    the TensorE contracts over the partition dim, which must be in_features
    for both operands, while in_features is the contiguous axis of both
    DRAM tensors.
  - each core computes its 1024-row slice of y = xT.T @ wqT with fp32
    accumulation in PSUM, streaming W (32 MiB fp16) once from HBM while the
    x shard (8 MiB fp16) stays SBUF-resident.  Loads are 256 KiB pieces
    chained into serial lanes in first-use order so the PE starts ~10us in
    and then streams 2048 N=512 matmuls back-to-back at ~217 ns each
    (hardware peak is ~216 ns: 512 cols / 2.4 GHz + NX issue overhead).
  - host concatenates the 8 row-slices.

Measured on trn2: ~462 us per core (roofline: 437 us of fp16 matmul),
2-norm relative error ~2.1e-4 vs the fp32 reference.
"""

import numpy as np

import concourse.mybir as mybir
import concourse.tile as tile
from concourse import bacc
from concourse.bass_utils import run_bass_kernel_spmd
from concourse.tile import add_dep_helper

N_CORES = 8
P = 128
IN_F = 4096
OUT_F = 4096
ROWS = 4 * 2048
ROWS_PER_CORE = ROWS // N_CORES      # 1024
I_TILES = IN_F // P                  # 32
O_BLK = 512
O_BLKS = OUT_F // O_BLK              # 8
S_TILES = ROWS_PER_CORE // P         # 8

F16 = mybir.dt.float16
F32 = mybir.dt.float32

_NC_CACHE = {}


def _build_nc(in_f=IN_F, out_f=OUT_F, rows_per_core=ROWS_PER_CORE):
    i_tiles = in_f // P
    o_blks = out_f // O_BLK
    s_tiles = rows_per_core // P
    xg = 8 if i_tiles % 8 == 0 else 1       # DMA split granularity
    wg = 4 if i_tiles % 4 == 0 else 1

    nc = bacc.Bacc(None, target_bir_lowering=False)
    xt = nc.dram_tensor("xt", (in_f, rows_per_core), F16, kind="ExternalInput")
    wt = nc.dram_tensor("wt", (in_f, out_f), F16, kind="ExternalInput")
    y = nc.dram_tensor("y", (rows_per_core, out_f), F32, kind="ExternalOutput")

    xt_v = xt.rearrange("(ih p) s -> p ih s", p=P)   # [128, i_tiles, rows]
    wt_v = wt.rearrange("(ih p) o -> p ih o", p=P)   # [128, i_tiles, out_f]
    y_v = y.rearrange("(st p) o -> st p o", p=P)     # [s_tiles, 128, out_f]

    wq = 2                                  # i-tiles per w quarter-DMA (256 KiB)
    w_quarters = i_tiles // wq
    LANES = 8

    with tile.TileContext(nc) as tc:
        with (
            tc.tile_pool(name="xp", bufs=1) as xp,
            tc.tile_pool(name="wp", bufs=2) as wp,
            tc.tile_pool(name="op", bufs=4) as op,
            tc.tile_pool(name="pp", bufs=1, space="PSUM") as pp,
        ):
            # --- startup pipelining -------------------------------------
            # DMAs issued together fair-share HBM bandwidth, so an unordered
            # prefetch makes the first matmul wait for everything (~35us).
            # Instead every load is a 256 KiB piece, chained into LANES
            # serial chains in exact first-use order; o-block 0 runs
            # i-outer across the 8 PSUM banks so the PE starts as soon as
            # the first pieces land and streams behind the DMA wavefront.
            lane_tails = [None] * LANES
            n_item = 0
            head_dma = None  # first critical piece; lane heads chain off it

            def chained_dma(dst, src):
                nonlocal n_item
                lane = n_item % LANES
                d = nc.scalar.dma_start(dst, src)
                dep = lane_tails[lane] if lane_tails[lane] is not None else head_dma
                if dep is not None:
                    add_dep_helper(d.ins, dep.ins, reason="load lane")
                lane_tails[lane] = d
                n_item += 1
                return d

            # per-i-tile x tiles; allocated up front, loaded in need order
            x_tiles = [
                xp.tile([P, rows_per_core], F16, tag=f"x{i}", name=f"x{i}")
                for i in range(i_tiles)
            ]

            # PE warm-up: ~8 dummy matmuls while the first loads are in
            # flight flip the HAM clock gate (1.2 -> 2.4 GHz takes ~3.4us
            # of sustained PE activity) so the real stream starts warm.
            dm = op.tile([P, O_BLK], F16, tag="warm", name="warm")
            nc.any.memset(dm, 0.0)
            dps = pp.tile([P, O_BLK], F32, tag="ps0", name="warmps")
            for _ in range(8):
                nc.tensor.matmul(dps, dm[:, :P], dm, start=True, stop=True)

            def load_w_quarter(w_tiles, q, osl, chained):
                wtile = wp.tile([P, wq, O_BLK], F16, tag=f"w{q}", name=f"w{q}")
                src = wt_v[:, q * wq : (q + 1) * wq, osl]
                if chained:
                    chained_dma(wtile, src)
                else:
                    nc.scalar.dma_start(wtile, src)
                w_tiles.append(wtile)

            for ob in range(o_blks):
                osl = slice(ob * O_BLK, (ob + 1) * O_BLK)
                w_tiles = []
                if ob == 0:
                    # Critical head: the first matmuls need only w[i=0] and
                    # the first half of x[i=0] — ship those two 128 KiB
                    # pieces alone at full bandwidth on nc.sync; everything
                    # else chains behind the w head in LANES serial lanes.
                    half = rows_per_core // 2
                    wtile = wp.tile([P, wq, O_BLK], F16, tag="w0", name="w0")
                    head_dma = nc.sync.dma_start(wtile[:, 0:1, :], wt_v[:, 0:1, osl])
                    nc.sync.dma_start(x_tiles[0][:, :half], xt_v[:, 0, :half])
                    w_tiles.append(wtile)
                    # rest of the head tiles, then pieces in first-use order
                    chained_dma(x_tiles[0][:, half:], xt_v[:, 0, half:])
                    chained_dma(wtile[:, 1:2, :], wt_v[:, 1:2, osl])
                    chained_dma(x_tiles[1], xt_v[:, 1, :])
                    for q in range(1, w_quarters):
                        load_w_quarter(w_tiles, q, osl, chained=True)
                        for i in (wq * q, wq * q + 1):
                            chained_dma(x_tiles[i], xt_v[:, i, :])
                elif ob == 1:
                    # keep feeding the lanes; arrives during ob0 compute
                    for q in range(w_quarters):
                        load_w_quarter(w_tiles, q, osl, chained=True)
                else:
                    # paced naturally by slot reuse (bufs=2 per tag)
                    for q in range(w_quarters):
                        load_w_quarter(w_tiles, q, osl, chained=False)

                if ob == 0:
                    # i-outer: all 8 s-tiles accumulate in parallel banks,
                    # consuming input pieces in arrival order
                    pss = [
                        pp.tile([P, O_BLK], F32, tag=f"ps{st}", name=f"ps0_{st}")
                        for st in range(s_tiles)
                    ]
                    for i in range(i_tiles):
                        for st in range(s_tiles):
                            nc.tensor.matmul(
                                pss[st],
                                x_tiles[i][:, st * P : (st + 1) * P],
                                w_tiles[i // wq][:, i % wq, :],
                                start=(i == 0),
                                stop=(i == i_tiles - 1),
                            )
                    for st in range(s_tiles):
                        o_sb = op.tile([P, O_BLK], F32)
                        nc.vector.tensor_copy(o_sb, pss[st])
                        nc.sync.dma_start(y_v[st, :, osl], o_sb)
                else:
                    for st in range(s_tiles):
                        ps = pp.tile([P, O_BLK], F32, tag=f"ps{st}")
                        for i in range(i_tiles):
                            nc.tensor.matmul(
                                ps,
                                x_tiles[i][:, st * P : (st + 1) * P],
                                w_tiles[i // wq][:, i % wq, :],
                                start=(i == 0),
                                stop=(i == i_tiles - 1),
                            )
                        o_sb = op.tile([P, O_BLK], F32)
                        nc.vector.tensor_copy(o_sb, ps)
                        nc.sync.dma_start(y_v[st, :, osl], o_sb)
    nc.finalize()
    return nc


def _get_nc():
    if "nc" not in _NC_CACHE:
        _NC_CACHE["nc"] = _build_nc()
    return _NC_CACHE["nc"]


def _prep_inputs(x, weight):
    x2 = np.ascontiguousarray(x, dtype=np.float32).reshape(ROWS, IN_F).astype(np.float16)
    wq = np.sign(weight.astype(np.float32)).astype(np.float16)
    wt = np.ascontiguousarray(wq.T)  # [in, out]
    in_maps = []
    for c in range(N_CORES):
        xs = np.ascontiguousarray(x2[c * ROWS_PER_CORE : (c + 1) * ROWS_PER_CORE].T)
        in_maps.append({"xt": xs, "wt": wt})
    return in_maps


def _run(x, weight, trace=False, trace_cores=None):
    in_maps = _prep_inputs(x, weight)
    res = run_bass_kernel_spmd(
        _get_nc(),
        in_maps,
        core_ids=list(range(N_CORES)),
        trace=trace,
        trace_cores=trace_cores,
    )
    out = np.concatenate([res.results[c]["y"] for c in range(N_CORES)], axis=0)
    return out.reshape(4, 2048, OUT_F), res


def _run_in_subprocess(x, weight):
    """Fallback for rare transient NRT device errors: a fresh process gets a
    fresh PJRT client, which empirically recovers where in-process retries
    cannot."""
    import os
    import subprocess
    import sys
    import tempfile

    d = tempfile.mkdtemp(prefix="bitlinear_retry_")
    xp, wp, op = (os.path.join(d, f) for f in ("x.npy", "w.npy", "out.npy"))
    np.save(xp, np.ascontiguousarray(x))
    np.save(wp, np.ascontiguousarray(weight))
    code = (
        "import importlib.util, numpy as np\n"
        f"spec = importlib.util.spec_from_file_location('kernel_sub', {__file__!r})\n"
        "m = importlib.util.module_from_spec(spec)\n"
        "spec.loader.exec_module(m)\n"
        f"out, _ = m._run(np.load({xp!r}), np.load({wp!r}))\n"
        f"np.save({op!r}, out)\n"
    )
    last = None
    for _ in range(3):
        r = subprocess.run([sys.executable, "-c", code], capture_output=True)
        if r.returncode == 0 and os.path.exists(op):
            return np.load(op)
        last = r
    raise RuntimeError(
        f"subprocess retries failed: {last.returncode}\n{last.stderr[-2000:].decode(errors='replace')}"
    )


def kernel(x, weight):
    try:
        out, _ = _run(x, weight, trace=False)
        return out
    except Exception:
        return _run_in_subprocess(x, weight)


# revision 18
# speedup vs baseline: 1.0055x; 1.0035x over previous
"""BitLinear-STE forward on 8 Trainium2 NeuronCores.

Reference computes y = x @ sign(W).T with x:(4,2048,4096) f32, W:(4096,4096) f32.
Forward-only, so the STE proxy reduces to a plain matmul against sign(W).

Strategy (data parallel over rows, per the sharding hint):
  - host: q = sign(W) cast to fp16 (exact, values are +-1) and transposed to
    [in, out]; x cast to fp16 (rounding error ~2e-4 on the output) and
    transposed per-core to [in, rows/8].  Transposes happen on host because
    the TensorE contracts over the partition dim, which must be in_features
    for both operands, while in_features is the contiguous axis of both
    DRAM tensors.
  - each core computes its 1024-row slice of y = xT.T @ wqT with fp32
    accumulation in PSUM, streaming W (32 MiB fp16) once from HBM while the
    x shard (8 MiB fp16) stays SBUF-resident.  Loads are 256 KiB pieces
    chained into serial lanes in first-use order so the PE starts ~10us in
    and then streams 2048 N=512 matmuls back-to-back at ~217 ns each
    (hardware peak is ~216 ns: 512 cols / 2.4 GHz + NX issue overhead).
  - host concatenates the 8 row-slices.

Measured on trn2: ~462 us per core (roofline: 437 us of fp16 matmul),
2-norm relative error ~2.1e-4 vs the fp32 reference.
"""

import numpy as np

import concourse.mybir as mybir
import concourse.tile as tile
from concourse import bacc
from concourse.bass_utils import run_bass_kernel_spmd
from concourse.tile import add_dep_helper

N_CORES = 8
P = 128
IN_F = 4096
OUT_F = 4096
ROWS = 4 * 2048
ROWS_PER_CORE = ROWS // N_CORES      # 1024
I_TILES = IN_F // P                  # 32
O_BLK = 512
O_BLKS = OUT_F // O_BLK              # 8
S_TILES = ROWS_PER_CORE // P         # 8

F16 = mybir.dt.float16
F32 = mybir.dt.float32

_NC_CACHE = {}


def _build_nc(in_f=IN_F, out_f=OUT_F, rows_per_core=ROWS_PER_CORE):
    i_tiles = in_f // P
    o_blks = out_f // O_BLK
    s_tiles = rows_per_core // P

    nc = bacc.Bacc(None, target_bir_lowering=False)
    xt = nc.dram_tensor("xt", (in_f, rows_per_core), F16, kind="ExternalInput")
    wt = nc.dram_tensor("wt", (in_f, out_f), F16, kind="ExternalInput")
    y = nc.dram_tensor("y", (rows_per_core, out_f), F32, kind="ExternalOutput")

    xt_v = xt.rearrange("(ih p) s -> p ih s", p=P)   # [128, i_tiles, rows]
    wt_v = wt.rearrange("(ih p) o -> p ih o", p=P)   # [128, i_tiles, out_f]
    y_v = y.rearrange("(st p) o -> st p o", p=P)     # [s_tiles, 128, out_f]

    wq = 2                                  # i-tiles per w quarter-DMA (256 KiB)
    w_quarters = i_tiles // wq
    LANES = 8

    with tile.TileContext(nc) as tc:
        with (
            tc.tile_pool(name="xp", bufs=1) as xp,
            tc.tile_pool(name="wp", bufs=2) as wp,
            tc.tile_pool(name="op", bufs=4) as op,
            tc.tile_pool(name="pp", bufs=1, space="PSUM") as pp,
        ):
            # --- startup pipelining -------------------------------------
            # DMAs issued together fair-share HBM bandwidth, so an unordered
            # prefetch makes the first matmul wait for everything (~35us).
            # Instead every load is a 256 KiB piece, chained into LANES
            # serial chains in exact first-use order; o-block 0 runs
            # i-outer across the 8 PSUM banks so the PE starts as soon as
            # the first pieces land and streams behind the DMA wavefront.
            lane_tails = [None] * LANES
            n_item = 0
            head_dma = None  # first critical piece; lane heads chain off it

            def chained_dma(dst, src):
                nonlocal n_item
                lane = n_item % LANES
                d = nc.scalar.dma_start(dst, src)
                dep = lane_tails[lane] if lane_tails[lane] is not None else head_dma
                if dep is not None:
                    add_dep_helper(d.ins, dep.ins, reason="load lane")
                lane_tails[lane] = d
                n_item += 1
                return d

            # per-i-tile x tiles; allocated up front, loaded in need order
            x_tiles = [
                xp.tile([P, rows_per_core], F16, tag=f"x{i}", name=f"x{i}")
                for i in range(i_tiles)
            ]

            # PE warm-up: ~8 dummy matmuls while the first loads are in
            # flight flip the HAM clock gate (1.2 -> 2.4 GHz takes ~3.4us
            # of sustained PE activity) so the real stream starts warm.
            dm = op.tile([P, O_BLK], F16, tag="warm", name="warm")
            nc.any.memset(dm, 0.0)
            dps = pp.tile([P, O_BLK], F32, tag="ps0", name="warmps")
            for _ in range(8):
                nc.tensor.matmul(dps, dm[:, :P], dm, start=True, stop=True)

            def load_w_quarter(w_tiles, q, osl, chained):
                wtile = wp.tile([P, wq, O_BLK], F16, tag=f"w{q}", name=f"w{q}")
                src = wt_v[:, q * wq : (q + 1) * wq, osl]
                if chained:
                    chained_dma(wtile, src)
                else:
                    nc.scalar.dma_start(wtile, src)
                w_tiles.append(wtile)

            for ob in range(o_blks):
                osl = slice(ob * O_BLK, (ob + 1) * O_BLK)
                w_tiles = []
                if ob == 0:
                    # Critical head: the first matmuls need only w[i=0] and
                    # the first half of x[i=0] — ship those two 128 KiB
                    # pieces alone at full bandwidth on nc.sync; everything
                    # else chains behind the w head in LANES serial lanes.
                    half = rows_per_core // 2
                    wtile = wp.tile([P, wq, O_BLK], F16, tag="w0", name="w0")
                    head_dma = nc.sync.dma_start(wtile[:, 0:1, :], wt_v[:, 0:1, osl])
                    nc.sync.dma_start(x_tiles[0][:, :half], xt_v[:, 0, :half])
                    w_tiles.append(wtile)
                    # rest of the head tiles, then pieces in first-use order
                    chained_dma(x_tiles[0][:, half:], xt_v[:, 0, half:])
                    chained_dma(wtile[:, 1:2, :], wt_v[:, 1:2, osl])
                    chained_dma(x_tiles[1], xt_v[:, 1, :])
                    for q in range(1, w_quarters):
                        load_w_quarter(w_tiles, q, osl, chained=True)
                        for i in (wq * q, wq * q + 1):
                            chained_dma(x_tiles[i], xt_v[:, i, :])
                elif ob == 1:
                    # keep feeding the lanes; arrives during ob0 compute
                    for q in range(w_quarters):
                        load_w_quarter(w_tiles, q, osl, chained=True)
                else:
                    # paced naturally by slot reuse (bufs=2 per tag)
                    for q in range(w_quarters):
                        load_w_quarter(w_tiles, q, osl, chained=False)

                if ob == 0:
                    # i-outer: all 8 s-tiles accumulate in parallel banks,
                    # consuming input pieces in arrival order
                    pss = [
                        pp.tile([P, O_BLK], F32, tag=f"ps{st}", name=f"ps0_{st}")
                        for st in range(s_tiles)
                    ]
                    for i in range(i_tiles):
                        for st in range(s_tiles):
                            nc.tensor.matmul(
                                pss[st],
                                x_tiles[i][:, st * P : (st + 1) * P],
                                w_tiles[i // wq][:, i % wq, :],
                                start=(i == 0),
                                stop=(i == i_tiles - 1),
                            )
                    for st in range(s_tiles):
                        o_sb = op.tile([P, O_BLK], F32)
                        nc.vector.tensor_copy(o_sb, pss[st])
                        nc.sync.dma_start(y_v[st, :, osl], o_sb)
                else:
                    for st in range(s_tiles):
                        ps = pp.tile([P, O_BLK], F32, tag=f"ps{st}")
                        for i in range(i_tiles):
                            nc.tensor.matmul(
                                ps,
                                x_tiles[i][:, st * P : (st + 1) * P],
                                w_tiles[i // wq][:, i % wq, :],
                                start=(i == 0),
                                stop=(i == i_tiles - 1),
                            )
                        o_sb = op.tile([P, O_BLK], F32)
                        nc.vector.tensor_copy(o_sb, ps)
                        nc.sync.dma_start(y_v[st, :, osl], o_sb)
    nc.finalize()
    return nc


def _get_nc():
    if "nc" not in _NC_CACHE:
        _NC_CACHE["nc"] = _build_nc()
    return _NC_CACHE["nc"]


def _prep_inputs(x, weight):
    x2 = np.ascontiguousarray(x, dtype=np.float32).reshape(ROWS, IN_F).astype(np.float16)
    wq = np.sign(weight.astype(np.float32)).astype(np.float16)
    wt = np.ascontiguousarray(wq.T)  # [in, out]
    in_maps = []
    for c in range(N_CORES):
        xs = np.ascontiguousarray(x2[c * ROWS_PER_CORE : (c + 1) * ROWS_PER_CORE].T)
        in_maps.append({"xt": xs, "wt": wt})
    return in_maps


def _run(x, weight, trace=False, trace_cores=None):
    in_maps = _prep_inputs(x, weight)
    res = run_bass_kernel_spmd(
        _get_nc(),
        in_maps,
        core_ids=list(range(N_CORES)),
        trace=trace,
        trace_cores=trace_cores,
    )
    out = np.concatenate([res.results[c]["y"] for c in range(N_CORES)], axis=0)
    return out.reshape(4, 2048, OUT_F), res


def _run_in_subprocess(x, weight):
    """Fallback for rare transient NRT device errors: a fresh process gets a
    fresh PJRT client, which empirically recovers where in-process retries
    cannot."""
    import os
    import subprocess
    import sys
    import tempfile

    d = tempfile.mkdtemp(prefix="bitlinear_retry_")
    xp, wp, op = (os.path.join(d, f) for f in ("x.npy", "w.npy", "out.npy"))
    np.save(xp, np.ascontiguousarray(x))
    np.save(wp, np.ascontiguousarray(weight))
    code = (
        "import importlib.util, numpy as np\n"
        f"spec = importlib.util.spec_from_file_location('kernel_sub', {__file__!r})\n"
        "m = importlib.util.module_from_spec(spec)\n"
        "spec.loader.exec_module(m)\n"
        f"out, _ = m._run(np.load({xp!r}), np.load({wp!r}))\n"
        f"np.save({op!r}, out)\n"
    )
    last = None
    for _ in range(3):
        r = subprocess.run(
            [sys.executable, "-c", code], capture_output=True, timeout=900
        )
        if r.returncode == 0 and os.path.exists(op):
            return np.load(op)
        last = r
    raise RuntimeError(
        f"subprocess retries failed: {last.returncode}\n{last.stderr[-2000:].decode(errors='replace')}"
    )


def kernel(x, weight):
    try:
        out, _ = _run(x, weight, trace=False)
        return out
    except Exception:
        return _run_in_subprocess(x, weight)
